# revision 1
# baseline (speedup 1.0000x reference)
"""CARAFE (scale=2, k_up=5) Trainium2 Bass kernel, data-parallel over batch on 8 cores.

Pipeline (all in low-res coordinates, per image):
  comp = SiLU(1x1conv(X) * inv1 + beta1)          # BN folded into weights host-side
  L    = 3x3conv(comp) * inv2 + beta2             # 100 channels, permuted to (g, k2)
  E    = exp(L)                                   # unnormalized softmax
  D    = group sums (indicator matmul), Dinv = exp(-ln D)
  P    = E * Dinv_broadcast                       # softmax probabilities
  out[c, 2i+di, 2j+dj] = sum_{ki,kj} P[(di*2+dj)*25 + ki*5+kj, i, j] * Xpad[c, i+ki-2, j+kj-2]

Stage 4 (v1): per tap, broadcast the P row to 128 partitions with a K=1 matmul
(ones stationary), then multiply-accumulate on the vector engine.
"""

import numpy as np

import concourse.bacc as bacc
import concourse.tile as tile
from concourse import mybir
import concourse.bass as bass

F32 = mybir.dt.float32
F32R = mybir.dt.float32r

SCALE, K_UP = 2, 5
EPS = 1e-5
B, C, H, W = 16, 256, 80, 80
CMID, NENC = 64, 100
NCORES = 8
BPC = B // NCORES  # images per core
HP = WP = H + 4    # padded 84x84 X/comp buffers

# row blocks: 13 x 6 rows + 1 x 2 rows
BLOCKS = [(i * 6, 6) for i in range(13)] + [(78, 2)]


# --------------------------------------------------------------------------
# host-side parameter prep
# --------------------------------------------------------------------------

def _prep_params(w_comp, g1, b1, m1, v1, w_enc, g2, b2, m2, v2):
    inv1 = (g1 / np.sqrt(v1 + EPS)).astype(np.float32)
    beta1 = (b1 - m1 * inv1).astype(np.float32)
    inv2 = (g2 / np.sqrt(v2 + EPS)).astype(np.float32)
    beta2 = (b2 - m2 * inv2).astype(np.float32)

    # comp 1x1 conv, BN scale folded: wcs[o, ci]
    wcs = (w_comp[:, :, 0, 0] * inv1[:, None]).astype(np.float32)  # [64, 256]
    # lhsT per input-channel chunk: [2][K=128, M=128], M = out channel dup'd (m%64)
    wcomp = np.zeros((2, 128, 128), np.float32)
    for ch in range(2):
        for mm in range(128):
            wcomp[ch, :, mm] = wcs[mm % 64, ch * 128:(ch + 1) * 128]
    beta1_dup = beta1[np.arange(128) % 64].reshape(128, 1).astype(np.float32)

    # enc 3x3 conv, BN folded, output channels permuted: device p = g*25 + k2,
    # original channel e = k2*4 + g  (g = di*2 + dj)
    wes = (w_enc * inv2[:, None, None, None]).astype(np.float32)  # [100, 64, 3, 3]
    # device channel p = 32*g + k2 (k2 < 25; partitions 32g+25..32g+31 are pad)
    # original channel e = k2*4 + g
    wes_p = np.zeros((128, 64, 3, 3), np.float32)
    beta2_p = np.zeros((128, 1), np.float32)
    for g in range(4):
        for k2 in range(25):
            wes_p[32 * g + k2] = wes[k2 * 4 + g]
            beta2_p[32 * g + k2, 0] = beta2[k2 * 4 + g]

    # wencA[a][K=128, M=128]: rows 0-63 tap (a, b=0), rows 64-127 tap (a, b=1)
    # wencB[a][K=64, M=128]: tap (a, b=2)
    wencA = np.zeros((3, 128, 128), np.float32)
    wencB = np.zeros((3, 64, 128), np.float32)
    for a in range(3):
        wencA[a, :64, :] = wes_p[:, :, a, 0].T
        wencA[a, 64:, :] = wes_p[:, :, a, 1].T
        wencB[a, :, :] = wes_p[:, :, a, 2].T

    indic = np.zeros((128, 4), np.float32)
    indicT = np.zeros((4, 128), np.float32)
    for g in range(4):
        for k2 in range(25):
            indic[32 * g + k2, g] = 1.0
            indicT[g, 32 * g + k2] = 1.0
    # sel[p, t, m] = 1 if p % 32 == t  (one-hot selector for the broadcast matmul)
    sel = np.zeros((128, 25, 128), np.float32)
    for p in range(128):
        if p % 32 < 25:
            sel[p, p % 32, :] = 1.0

    return dict(
        wcomp=wcomp, beta1=beta1_dup, wencA=wencA, wencB=wencB,
        beta2=beta2_p, indic=indic, indicT=indicT, sel=sel,
    )


# --------------------------------------------------------------------------
# device program (SPMD; identical on all 8 cores)
# --------------------------------------------------------------------------

def _build_nc(reps=1):
    nc = bacc.Bacc(None, target_bir_lowering=False, debug=False)
    AF = mybir.ActivationFunctionType

    x_d = nc.declare_dram_parameter("x", [BPC, 2, 128, H, W], F32R, isOutput=False)
    wcomp_d = nc.declare_dram_parameter("wcomp", [2, 128, 128], F32R, isOutput=False)
    beta1_d = nc.declare_dram_parameter("beta1", [128, 1], F32, isOutput=False)
    wencA_d = nc.declare_dram_parameter("wencA", [3, 128, 128], F32R, isOutput=False)
    wencB_d = nc.declare_dram_parameter("wencB", [3, 64, 128], F32R, isOutput=False)
    beta2_d = nc.declare_dram_parameter("beta2", [128, 1], F32, isOutput=False)
    indic_d = nc.declare_dram_parameter("indic", [128, 4], F32R, isOutput=False)
    indicT_d = nc.declare_dram_parameter("indicT", [4, 128], F32R, isOutput=False)
    sel_d = nc.declare_dram_parameter("sel", [128, 25, 128], F32R, isOutput=False)
    # out rows are stored [i, di] so no strided slicing is needed
    out_d = nc.declare_dram_parameter("out", [BPC, C, H, 2, 2 * W], F32, isOutput=True)

    with tile.TileContext(nc) as tc:
        with (
            tc.tile_pool(name="const", bufs=1) as cpool,
            tc.tile_pool(name="xbuf", bufs=1) as xpool,
            tc.tile_pool(name="ebuf", bufs=1) as epool,
            tc.tile_pool(name="big", bufs=1) as bigpool,
            tc.tile_pool(name="tmp", bufs=3) as tpool,
            tc.tile_pool(name="psA", bufs=2, space="PSUM") as psA,
            tc.tile_pool(name="psP", bufs=2, space="PSUM") as psP,
            tc.tile_pool(name="psD", bufs=1, space="PSUM") as psD,
        ):
            # ---- constants (partition dim is the first dim of an SBUF tile) ----
            wcomp_sb = [cpool.tile([128, 128], F32R, name=f"wcomp{i}") for i in range(2)]
            wencA_sb = [cpool.tile([128, 128], F32R, name=f"wencA{i}") for i in range(3)]
            wencB_sb = [cpool.tile([64, 128], F32R, name=f"wencB{i}") for i in range(3)]
            beta1_sb = cpool.tile([128, 1], F32)
            beta2_sb = cpool.tile([128, 1], F32)
            indic_sb = cpool.tile([128, 4], F32R)
            indicT_sb = cpool.tile([4, 128], F32R)
            sel_sb = cpool.tile([128, 25, 128], F32R)
            for i in range(2):
                nc.sync.dma_start(wcomp_sb[i][:], wcomp_d[i])
            for a in range(3):
                nc.sync.dma_start(wencA_sb[a][:], wencA_d[a])
                nc.sync.dma_start(wencB_sb[a][:], wencB_d[a])
            nc.sync.dma_start(beta1_sb[:], beta1_d[:])
            nc.sync.dma_start(beta2_sb[:], beta2_d[:])
            nc.sync.dma_start(indic_sb[:], indic_d[:])
            nc.sync.dma_start(indicT_sb[:], indicT_d[:])
            nc.sync.dma_start(sel_sb[:], sel_d[:])

            # ---- persistent buffers ----
            x_sb = [xpool.tile([128, HP, WP], F32R, name=f"x{ch}", tag=f"x{ch}") for ch in range(2)]
            comp_sb = bigpool.tile([128, HP, WP], F32R, tag="bigslot")
            e_sb = epool.tile([128, H, W], F32R)
            d_sb = epool.tile([4, H, W], F32R)
            for ch in range(2):
                nc.vector.memset(
                    x_sb[ch].rearrange("p h w -> p (h w)").bitcast(F32), 0.0)
            nc.vector.memset(comp_sb.rearrange("p h w -> p (h w)").bitcast(F32), 0.0)

            for img in range(BPC * reps):
                img = img % BPC
                # ---- load X (interior of padded buffer) ----
                for ch in range(2):
                    nc.sync.dma_start(x_sb[ch][:, 2:82, 2:82], x_d[img, ch])

                # ---- comp: 1x1 conv + SiLU ----
                for (r0, nr) in BLOCKS:
                    n = nr * W
                    ps = psA.tile([128, 480], F32, tag="psA")
                    nc.tensor.matmul(
                        ps[:, :n], wcomp_sb[0][:], x_sb[0][:, 2 + r0:2 + r0 + nr, 2:82],
                        start=True, stop=False)
                    nc.tensor.matmul(
                        ps[:, :n], wcomp_sb[1][:], x_sb[1][:, 2 + r0:2 + r0 + nr, 2:82],
                        start=False, stop=True)
                    # lower half: comp(i,j) at col 2+j ; upper half shifted: at col 1+j
                    nc.scalar.activation(
                        comp_sb[0:64, 2 + r0:2 + r0 + nr, 2:82], ps[0:64, :n],
                        AF.Silu, bias=beta1_sb[0:64])
                    nc.scalar.activation(
                        comp_sb[64:128, 2 + r0:2 + r0 + nr, 1:81], ps[64:128, :n],
                        AF.Silu, bias=beta1_sb[64:128])

                # ---- enc: 3x3 conv + exp ----
                for (r0, nr) in BLOCKS:
                    n = nr * W
                    ps = psA.tile([128, 480], F32, tag="psA")
                    for ri in range(3):
                        r = ri - 1
                        nc.tensor.matmul(
                            ps[:, :n], wencA_sb[ri][:],
                            comp_sb[:, 2 + r0 + r:2 + r0 + r + nr, 1:81],
                            start=(ri == 0), stop=False)
                        nc.tensor.matmul(
                            ps[:, :n], wencB_sb[ri][:],
                            comp_sb[0:64, 2 + r0 + r:2 + r0 + r + nr, 3:83],
                            start=False, stop=(ri == 2))
                    nc.scalar.activation(
                        e_sb[:, r0:r0 + nr, :], ps[:, :n], AF.Exp, bias=beta2_sb[:])

                # ---- softmax denominators: D = group sums, Dinv = exp(-ln D) ----
                for (r0, nr) in BLOCKS:
                    n = nr * W
                    ps = psD.tile([4, 480], F32, tag="psD")
                    nc.tensor.matmul(ps[:, :n], indic_sb[:], e_sb[:, r0:r0 + nr, :])
                    nc.scalar.activation(d_sb[:, r0:r0 + nr, :], ps[:, :n], AF.Ln)
                nc.scalar.activation(d_sb[:], d_sb[:], AF.Exp, scale=-1.0)
                # fold into E -> P
                for (r0, nr) in BLOCKS:
                    n = nr * W
                    ps = psD.tile([128, 480], F32, tag="psD2")
                    nc.tensor.matmul(ps[:, :n], indicT_sb[:], d_sb[:, r0:r0 + nr, :])
                    nc.vector.tensor_mul(
                        e_sb[:, r0:r0 + nr, :], e_sb[:, r0:r0 + nr, :], ps[:, :n])

                # ---- stage 4: dynamic 5x5 filter (12-row blocks) ----
                S4B = [(i * 12, 12) for i in range(6)] + [(72, 6), (78, 2)]
                for di in range(2):
                    for ch in range(2):
                        for (r0, nr) in S4B:
                            nh = nr // 2 if nr == 12 else nr  # rows per half
                            n = nh * W
                            halves = 2 if nr == 12 else 1
                            # per-block output accumulator [c, i, j, dj]
                            ab = tpool.tile([128, 12, W, 2], F32, tag="accblk")
                            for dj in range(2):
                                g = di * 2 + dj
                                dst = ab[:, :nr, :, dj]
                                for t in range(25):
                                    ki, kj = t // 5, t % 5
                                    pb = psP.tile([128, 2, 512], F32, tag="psP")
                                    for b in range(halves):
                                        nc.tensor.matmul(
                                            pb[:, b, :n],
                                            sel_sb[32 * g:32 * g + 32, t, :],
                                            e_sb[32 * g:32 * g + 32,
                                                 r0 + b * nh:r0 + b * nh + nh, :],
                                            tile_position=(32 * g, 0))
                                    xs = x_sb[ch][:, r0 + ki:r0 + ki + nr, kj:kj + 80]
                                    pbv = pb[:, :halves, :n]
                                    if t == 0:
                                        nc.vector.tensor_mul(dst, xs, pbv)
                                    else:
                                        tmp = tpool.tile([128, 1024], F32, tag="tmp")
                                        nc.vector.tensor_mul(tmp[:, :nr * W], xs, pbv)
                                        nc.vector.tensor_add(dst, dst, tmp[:, :nr * W])
                            nc.sync.dma_start(
                                out_d[img, ch * 128:(ch + 1) * 128, r0:r0 + nr, di, :],
                                ab[:, :nr, :, :])
    nc.compile()
    return nc


# --------------------------------------------------------------------------
# runner
# --------------------------------------------------------------------------

_CACHE = {}


def _get_nc(reps=1):
    key = f"nc{reps}"
    if key not in _CACHE:
        _CACHE[key] = _build_nc(reps)
    return _CACHE[key]


def make_in_maps(X, w_comp, g1, b1, m1, v1, w_enc, g2, b2, m2, v2):
    params = _prep_params(np.asarray(w_comp), np.asarray(g1), np.asarray(b1),
                          np.asarray(m1), np.asarray(v1), np.asarray(w_enc),
                          np.asarray(g2), np.asarray(b2), np.asarray(m2),
                          np.asarray(v2))
    X = np.asarray(X, np.float32).reshape(NCORES, BPC, 2, 128, H, W)
    return [dict(params, x=np.ascontiguousarray(X[i])) for i in range(NCORES)]


def _get_runner():
    """Sharded 8-core PJRT callable, built once (modeled on run_bass_via_pjrt)."""
    if "runner" in _CACHE:
        return _CACHE["runner"]
    import jax
    from jax.experimental.shard_map import shard_map
    from jax.sharding import Mesh, PartitionSpec
    from concourse import bass2jax, mybir as mb

    nc = _get_nc()
    bass2jax.install_neuronx_cc_hook()
    assert nc.dbg_addr is None
    pid_name = nc.partition_id_tensor.name if nc.partition_id_tensor else None

    in_names, out_names, out_avals = [], [], []
    for alloc in nc.m.functions[0].allocations:
        if not isinstance(alloc, mb.MemoryLocationSet):
            continue
        name = alloc.memorylocations[0].name
        if alloc.kind == "ExternalInput":
            if name != pid_name:
                in_names.append(name)
        elif alloc.kind == "ExternalOutput":
            out_names.append(name)
            out_avals.append(jax.core.ShapedArray(
                tuple(alloc.tensor_shape), mb.dt.np(alloc.dtype)))
    n_params = len(in_names)
    zero_outs = [np.zeros(a.shape, a.dtype) for a in out_avals]
    all_names = in_names + out_names
    if pid_name is not None:
        all_names = all_names + [pid_name]

    def _body(*args):
        operands = list(args)
        if pid_name is not None:
            operands.append(bass2jax.partition_id_tensor())
        outs = bass2jax._bass_exec_p.bind(
            *operands,
            out_avals=tuple(out_avals),
            in_names=tuple(all_names),
            out_names=tuple(out_names),
            lowering_input_output_aliases=(),
            sim_require_finite=True,
            sim_require_nnan=True,
            nc=nc,
        )
        return tuple(outs)

    devices = jax.devices()[:NCORES]
    mesh = Mesh(np.asarray(devices), ("core",))
    n_out = len(out_names)
    donate = tuple(range(n_params, n_params + n_out))
    sharded = jax.jit(shard_map(
        _body, mesh=mesh,
        in_specs=(PartitionSpec("core"),) * (n_params + n_out),
        out_specs=(PartitionSpec("core"),) * n_out,
        check_rep=False), donate_argnums=donate, keep_unused=True)

    concat_zeros = [np.zeros((NCORES * z.shape[0], *z.shape[1:]), z.dtype)
                    for z in zero_outs]
    runner = dict(fn=sharded, in_names=in_names, out_names=out_names,
                  out_avals=out_avals, zeros=concat_zeros)
    _CACHE["runner"] = runner
    return runner


def _run(in_maps):
    r = _get_runner()
    concat_in = [np.concatenate([np.asarray(m[n]) for m in in_maps], axis=0)
                 for n in r["in_names"]]
    outs = r["fn"](*concat_in, *r["zeros"])
    return outs


def kernel(X, w_comp, g1, b1, m1, v1, w_enc, g2, b2, m2, v2):
    in_maps = make_in_maps(X, w_comp, g1, b1, m1, v1, w_enc, g2, b2, m2, v2)
    r = _get_runner()
    outs = _run(in_maps)
    i = r["out_names"].index("out")
    full = np.asarray(outs[i]).reshape(NCORES, BPC, C, 2 * H, 2 * W)
    return full.reshape(B, C, 2 * H, 2 * W).astype(np.float32)


def time_exec(inputs, iters=5):
    """Wall-time the sharded call with device-resident inputs."""
    import time
    import jax

    in_maps = make_in_maps(**inputs)
    r = _get_runner()
    concat_in = [np.concatenate([np.asarray(m[n]) for m in in_maps], axis=0)
                 for n in r["in_names"]]
    ins = [jax.device_put(a) for a in concat_in]
    jax.block_until_ready(ins)
    times = []
    for i in range(iters + 1):
        zs = [jax.device_put(z) for z in r["zeros"]]
        jax.block_until_ready(zs)
        t0 = time.perf_counter()
        out = r["fn"](*ins, *zs)
        jax.block_until_ready(out)
        dt = time.perf_counter() - t0
        if i > 0:  # skip warmup
            times.append(dt)
    return min(times)

